# revision 1
# baseline (speedup 1.0000x reference)
"""ODE-RNN on Trainium2 (Bass/Tile), data-parallel over batch on 8 NeuronCores.

Strategy (per core, batch slice of 32, everything SBUF-resident):
  - h kept transposed: h_sb[p, 32k+b] = h[b, 128k+p]  ([128, 256] f16)
  - weights host-pretransposed+tiled so stationary tile (k,m) is
    w_sb[:, (k*8+m)*128 : +128] and psum[:, m-group] += tile.T @ h_k
  - U = x @ W_in.T (+ b_in + b_h) precomputed on-device for all timesteps
    (h-independent), consumed per-step by the RNN update
  - timestep 0 shortcut: dts[0] == 0 and h0 == 0 imply h after step 0 is
    tanh(U_0) for any inputs, so its 5 matmul blocks are skipped
  - fp16 operands with fp32 PSUM accumulation; all-zero-bias fast path
    (build_fast, v5 PSUM-resident z recurrence — see its docstring),
    general path with biases / non-uniform dt (build)

The recurrent phase is weight-load bound: each [128,128] f16 stationary
tile load sustains ~25ns back-to-back with its N=32 matmul, so a 64-MM
block floors at ~1.6us and the whole kernel at ~540us. v5 reaches toward
that floor by keeping tanh the only op on the cross-block critical path
and staggering psum-bank completion so ACT overlaps the next block.
"""

import sys

import numpy as np

B, S, I, H, N_ODE = 256, 64, 256, 1024, 4
NCORES = 8
BL = B // NCORES  # 32
KT = H // 128  # 8
KI = I // 128  # 2


def legalize_sync_waits(nc, max_waits=1):
    """This container's walrus rejects instructions carrying more than one
    sync-wait ("Too many sync wait commands", setupSyncWait). Hoist excess
    waits onto same-engine nop carriers inserted right before the offender."""
    n_split = 0
    for f in nc.m.functions:
        for bb in f.blocks:
            lst = bb.instructions
            i = 0
            while i < len(lst):
                inst = lst[i]
                si = inst.sync_info
                waits = list(si.on_wait) if (si and si.on_wait) else []
                if len(waits) > max_waits:
                    n_split += 1
                    keep = waits[-max_waits:]
                    hoist = waits[:-max_waits]
                    si.on_wait = keep
                    inst.sync_info = si
                    for w in hoist:
                        nop = nc.engines[inst.engine].nop(nofuse=True)
                        nsi = nop.ins.sync_info
                        if nsi is None:
                            import bass_rust
                            nsi = bass_rust.SyncInfo(on_wait=[w], on_update=[])
                        else:
                            nsi.on_wait = [w]
                        nop.ins.sync_info = nsi
                        # emission appended it to nc.cur_bb's list; relocate
                        src = nc.cur_bb.bb.instructions
                        assert src[-1].name == nop.ins.name
                        src.pop()
                        lst.insert(i, nop.ins)
                        i += 1
                i += 1
    return n_split


def build(dths, n_steps=S, with_bias=False):
    """Build the Bass module. dths: list of n_steps floats (dt/N_ODE per step)."""
    import concourse.bass as bass
    import concourse.tile as tile
    from concourse import mybir

    f16 = mybir.dt.float16
    f32 = mybir.dt.float32
    Tanh = mybir.ActivationFunctionType.Tanh
    Ident = mybir.ActivationFunctionType.Identity
    mult = mybir.AluOpType.mult
    add = mybir.AluOpType.add

    nc = bass.Bass("TRN2", target_bir_lowering=False, debug=False)

    wo_d = nc.dram_tensor("wo", [128, KT * KT * 128], f16, kind="ExternalInput").ap()
    wh_d = nc.dram_tensor("wh", [128, KT * KT * 128], f16, kind="ExternalInput").ap()
    wi_d = nc.dram_tensor("wi", [128, KI * KT * 128], f16, kind="ExternalInput").ap()
    xt_d = nc.dram_tensor("xt", [128, KI * S * BL], f16, kind="ExternalInput").ap()
    out_d = nc.dram_tensor("hout", [128, KT * BL], f32, kind="ExternalOutput").ap()
    if with_bias:
        bode_d = nc.dram_tensor("bode", [128, KT * BL], f32, kind="ExternalInput").ap()
        binh_d = nc.dram_tensor("binh", [128, KT], f32, kind="ExternalInput").ap()

    wo_sb = nc.alloc_sbuf_tensor("wo_sb", [128, KT * KT * 128], f16).ap()
    wh_sb = nc.alloc_sbuf_tensor("wh_sb", [128, KT * KT * 128], f16).ap()
    wi_sb = nc.alloc_sbuf_tensor("wi_sb", [128, KI * KT * 128], f16).ap()
    xt_sb = nc.alloc_sbuf_tensor("xt_sb", [128, KI * S * BL], f16).ap()
    u_sb = nc.alloc_sbuf_tensor("u_sb", [128, KT * S * BL], f16).ap()
    hA = nc.alloc_sbuf_tensor("hA", [128, KT * BL], f16).ap()
    hB = nc.alloc_sbuf_tensor("hB", [128, KT * BL], f16).ap()
    hF = nc.alloc_sbuf_tensor("hF", [128, KT * BL], f32).ap()
    if with_bias:
        bode_sb = nc.alloc_sbuf_tensor("bode_sb", [128, KT * BL], f32).ap()
        binh_sb = nc.alloc_sbuf_tensor("binh_sb", [128, KT], f32).ap()

    SB = S * BL  # 2048 (s,b) columns per k2

    with tile.TileContext(nc) as tc:
        with (
            tc.tile_pool(name="ppre", bufs=2, space="PSUM") as ppre,
            tc.tile_pool(name="pmain", bufs=4, space="PSUM") as pmain,
            tc.tile_pool(name="ptz", bufs=3) as ptz,
            tc.tile_pool(name="pz", bufs=3) as pz,
        ):
            # input DMAs (precompute deps first)
            nc.sync.dma_start(wi_sb, wi_d)
            nc.sync.dma_start(xt_sb, xt_d)
            if with_bias:
                nc.sync.dma_start(binh_sb, binh_d)
                nc.sync.dma_start(bode_sb, bode_d)
            nc.sync.dma_start(wo_sb, wo_d)
            nc.sync.dma_start(wh_sb, wh_d)

            # --- U precompute: U = x @ W_in.T (+ b_in + b_h) over all (s,b) ---
            NCHUNK = 4
            CW = SB // NCHUNK  # 512
            for m in range(KT):
                for c in range(NCHUNK):
                    ps = ppre.tile([128, CW], f32)
                    for k2 in range(KI):
                        nc.tensor.matmul(
                            ps[:, :],
                            lhsT=wi_sb[:, (k2 * KT + m) * 128:(k2 * KT + m) * 128 + 128],
                            rhs=xt_sb[:, k2 * SB + c * CW: k2 * SB + (c + 1) * CW],
                            start=(k2 == 0),
                            stop=(k2 == KI - 1),
                        )
                    dst = u_sb[:, m * SB + c * CW: m * SB + (c + 1) * CW]
                    if with_bias:
                        if (m * NCHUNK + c) % 2 == 0:
                            nc.scalar.activation(dst, ps[:, :], Ident, bias=binh_sb[:, m:m + 1])
                        else:
                            nc.vector.tensor_scalar_add(dst, ps[:, :], binh_sb[:, m:m + 1])
                    else:
                        if (m * NCHUNK + c) % 2 == 0:
                            nc.scalar.copy(dst, ps[:, :])
                        else:
                            nc.vector.tensor_copy(dst, ps[:, :])

            # --- timestep 0: dts[0]=0 and h0=0  =>  h1 = tanh(U_0) ---
            u0 = u_sb.rearrange("p (m s b) -> p m (s b)", m=KT, s=S)[:, :, 0:BL]
            hA3 = hA.rearrange("p (m b) -> p m b", m=KT)
            if n_steps == 1:
                nc.scalar.activation(hF.rearrange("p (m b) -> p m b", m=KT), u0, Tanh)
            else:
                nc.scalar.activation(hA3, u0, Tanh)

            cur, nxt = hA, hB

            def mm_block(ps, w_sb, hin):
                for m in range(KT):
                    for k in range(KT):
                        nc.tensor.matmul(
                            ps[:, m * BL:(m + 1) * BL],
                            lhsT=w_sb[:, (k * KT + m) * 128:(k * KT + m) * 128 + 128],
                            rhs=hin[:, k * BL:(k + 1) * BL],
                            start=(k == 0),
                            stop=(k == KT - 1),
                        )

            for s in range(1, n_steps):
                dth = float(dths[s])
                # 4 Euler sub-steps
                for e in range(N_ODE):
                    ps = pmain.tile([128, KT * BL], f32)
                    mm_block(ps, wo_sb, cur)
                    tz = ptz.tile([128, KT * BL], f16)
                    if with_bias:
                        zb = pz.tile([128, KT * BL], f32)
                        nc.vector.tensor_add(zb[:, :], ps[:, :], bode_sb)
                        nc.scalar.activation(tz[:, :], zb[:, :], Tanh)
                    else:
                        nc.scalar.activation(tz[:, :], ps[:, :], Tanh)
                    nc.vector.scalar_tensor_tensor(nxt, tz[:, :], dth, cur, op0=mult, op1=add)
                    cur, nxt = nxt, cur
                # RNN update
                ps = pmain.tile([128, KT * BL], f32)
                mm_block(ps, wh_sb, cur)
                z = pz.tile([128, KT * BL], f32)
                us = u_sb.rearrange("p (m s b) -> p m s b", m=KT, s=S)[:, :, s, :]
                nc.vector.tensor_add(
                    z.rearrange("p (m b) -> p m b", m=KT), ps.rearrange("p (m b) -> p m b", m=KT), us
                )
                if s == n_steps - 1:
                    nc.scalar.activation(hF, z[:, :], Tanh)
                else:
                    nc.scalar.activation(nxt, z[:, :], Tanh)
                    cur, nxt = nxt, cur

            nc.sync.dma_start(out_d, hF)

    n_split = legalize_sync_waits(nc)
    print(f"legalize_sync_waits: split {n_split} instructions")
    return nc


# MM emission priorities (m, k) per Euler sub-step, plus the RNN block's.
# The Tile scheduler re-sorts by virtual-cost readiness (its matmul cost
# omits LDWEIGHTS so the virtual PE runs ~2x fast); these priorities were
# tuned against a steady-state pipeline model and measured best on HW
# (748.8us total vs 766.8 baseline; plain m-outer 750.4, TILE_SCHEDULER=
# asap with in-order-optimal orders 1025.6 - asap convoys the pipeline).
# MM emission priorities (m, k) per Euler sub-step, plus the RNN block's.
# The Tile scheduler re-sorts by virtual-cost readiness (its matmul cost
# omits LDWEIGHTS so the virtual PE runs ~2x fast); these priorities were
# tuned against a steady-state pipeline model and measured best on HW
# (747.8us total vs 766.8 baseline; plain m-outer 750.4; 2-half-tanh
# granularity 795.3; TILE_SCHEDULER=asap with in-order-optimal orders
# 1025.6 - asap convoys the pipeline).
_EO = [(0,0),(1,0),(3,2),(3,3),(3,1),(0,1),(3,0),(1,1),(2,1),(2,0),(0,2),(2,3),(0,3),(2,2),(1,2),(4,5),(5,0),(5,4),(5,3),(3,5),(0,4),(2,4),(0,5),(1,5),(1,4),(2,5),(1,3),(1,7),(0,7),(0,6),(1,6),(4,4),(3,6),(5,2),(2,7),(6,5),(2,6),(7,0),(6,0),(6,2),(3,4),(5,1),(3,7),(7,1),(4,0),(7,7),(5,7),(5,5),(6,3),(4,7),(4,6),(7,3),(4,1),(4,3),(5,6),(4,2),(7,5),(7,6),(7,2),(6,1),(6,4),(6,7),(6,6),(7,4)]
EORDERS = [_EO, _EO, _EO, _EO]
RORDER = [(3,1),(3,0),(0,1),(0,0),(5,1),(2,1),(1,1),(5,0),(4,0),(4,1),(0,2),(2,2),(6,1),(6,0),(0,3),(1,0),(7,0),(7,3),(1,2),(2,3),(3,3),(1,3),(2,5),(4,4),(2,4),(1,4),(0,4),(7,2),(6,4),(3,2),(0,5),(1,5),(3,5),(3,4),(0,7),(1,6),(0,6),(1,7),(2,0),(3,6),(7,5),(5,5),(5,6),(3,7),(6,5),(4,5),(4,6),(2,6),(5,7),(2,7),(4,2),(5,4),(6,3),(6,6),(7,6),(7,1),(6,2),(5,3),(4,7),(4,3),(5,2),(7,7),(6,7),(7,4)]
ACT_E = [0, 1, 2, 3]
ACT_R = [0, 1, 2, 3]


def build_fast(dths, n_steps=S):
    """v5 zero-bias fast path: PSUM-resident z recurrence.

    Per timestep, the Euler chain is reassociated so PSUM accumulates z
    across sub-steps: z_0 = W_ode h_s; z_{e+1} = z_e + (dth*W_ode) t_e with
    t_e = tanh(z_e) read straight out of PSUM by ACT. This removes the DVE
    axpy from the critical path (h_4 = h_s + dth*sum(t_e) accumulates on DVE
    off-path). dth*W_ode is scaled on-chip (DVE) from the DMA'd W_ode.
    MMs are emitted in EORDER/RORDER so psum-bank completions stagger and
    each quarter's tanh overlaps the next block's matmuls. The RNN update
    injects U_s with a single N=256 identity matmul, then accumulates
    W_h h_4 on top. U precompute is chunk-major against split xt DMAs so
    the PE starts as soon as the first x chunk lands.
    """
    import os

    # Use the default CoreSim-cost Tile scheduler. (TILE_SCHEDULER=asap and
    # in-order-optimized emission were tried and measured SLOWER on HW:
    # asap re-derives its own earliest-feasible order from the same virtual
    # cost model and convoys the tanh/matmul pipeline.)
    os.environ.pop("TILE_SCHEDULER", None)

    import concourse.bass as bass
    import concourse.tile as tile
    from concourse import mybir

    f16 = mybir.dt.float16
    f32 = mybir.dt.float32
    Tanh = mybir.ActivationFunctionType.Tanh
    mult = mybir.AluOpType.mult
    add = mybir.AluOpType.add

    dth = float(dths[1]) if n_steps > 1 else 0.0

    nc = bass.Bass("TRN2", target_bir_lowering=False, debug=False)

    wo_d = nc.dram_tensor("wo", [128, KT * KT * 128], f16, kind="ExternalInput").ap()
    wh_d = nc.dram_tensor("wh", [128, KT * KT * 128], f16, kind="ExternalInput").ap()
    wi_d = nc.dram_tensor("wi", [128, KI * KT * 128], f16, kind="ExternalInput").ap()
    xt_d = nc.dram_tensor("xt", [128, KI * S * BL], f16, kind="ExternalInput").ap()
    id_d = nc.dram_tensor("ident", [128, 128], f16, kind="ExternalInput").ap()
    out_d = nc.dram_tensor("hout", [128, KT * BL], f32, kind="ExternalOutput").ap()

    wo_sb = nc.alloc_sbuf_tensor("wo_sb", [128, KT * KT * 128], f16).ap()
    wo2_sb = nc.alloc_sbuf_tensor("wo2_sb", [128, KT * KT * 128], f16).ap()
    wh_sb = nc.alloc_sbuf_tensor("wh_sb", [128, KT * KT * 128], f16).ap()
    wi_sb = nc.alloc_sbuf_tensor("wi_sb", [128, KI * KT * 128], f16).ap()
    xt_sb = nc.alloc_sbuf_tensor("xt_sb", [128, KI * S * BL], f16).ap()
    id_sb = nc.alloc_sbuf_tensor("id_sb", [128, 128], f16).ap()
    u_sb = nc.alloc_sbuf_tensor("u_sb", [128, KT * S * BL], f16).ap()
    z0_sb = nc.alloc_sbuf_tensor("z0_sb", [128, KT * BL], f16).ap()
    hF = nc.alloc_sbuf_tensor("hF", [128, KT * BL], f32).ap()

    SB = S * BL  # 2048
    W = KT * BL  # 256
    QW = W // 4  # 64
    QS = [slice(q * QW, (q + 1) * QW) for q in range(4)]

    # PSUM start=True zeroes the whole 2KB bank region (ZERO_REGION), so a
    # per-slice start flag clobbers sibling slices' accumulation. Instead each
    # psum z tile is initialized by ONE full-width start=True matmul (identity
    # x zeros / identity x U_s) and every slice matmul accumulates start=False.
    e_last = {m: max(i for i, (mm, _) in enumerate(EORDERS[N_ODE - 1]) if mm == m) for m in range(KT)}
    r_last = {m: max(i for i, (mm, _) in enumerate(RORDER) if mm == m) for m in range(KT)}

    with tile.TileContext(nc) as tc:
        with (
            tc.tile_pool(name="pt", bufs=6) as pt,
            tc.tile_pool(name="ph", bufs=4) as ph,
            tc.tile_pool(name="pa", bufs=6) as pa,
        ):
            nc.sync.dma_start(wi_sb, wi_d)
            # xt split chunk-major so U MMs start after the first slice lands
            NCHUNK = 4
            CW = SB // NCHUNK  # 512
            for c in range(NCHUNK):
                for k2 in range(KI):
                    off = k2 * SB + c * CW
                    nc.sync.dma_start(xt_sb[:, off:off + CW], xt_d[:, off:off + CW])
            nc.sync.dma_start(id_sb, id_d)
            nc.sync.dma_start(wo_sb, wo_d)
            nc.sync.dma_start(wh_sb, wh_d)
            nc.vector.memset(z0_sb, 0.0)

            # --- U = x @ W_in.T for all (s, b), chunk-major ---
            # wo2 = dth * wo scaling (DVE) is emitted mid-U: with in-order
            # (asap) emission, DVE ops placed earlier would block U's vector
            # copies behind the late wo DMA.
            NSC = 8
            SCW = KT * KT * 128 // NSC
            with tc.tile_pool(name="ppre", bufs=2, space="PSUM") as ppre:
                for c in range(NCHUNK):
                    if c == 2:
                        for j in range(NSC):
                            nc.vector.tensor_scalar_mul(
                                wo2_sb[:, j * SCW:(j + 1) * SCW], wo_sb[:, j * SCW:(j + 1) * SCW], dth
                            )
                    for m in range(KT):
                        ps = ppre.tile([128, CW], f32)
                        for k2 in range(KI):
                            nc.tensor.matmul(
                                ps[:, :],
                                lhsT=wi_sb[:, (k2 * KT + m) * 128:(k2 * KT + m) * 128 + 128],
                                rhs=xt_sb[:, k2 * SB + c * CW: k2 * SB + (c + 1) * CW],
                                start=(k2 == 0),
                                stop=(k2 == KI - 1),
                            )
                        dst = u_sb[:, m * SB + c * CW: m * SB + (c + 1) * CW]
                        if c >= 2 or (c * KT + m) % 2 == 0:
                            nc.scalar.copy(dst, ps[:, :])
                        else:
                            nc.vector.tensor_copy(dst, ps[:, :])

            # --- timestep 0: dts[0] == 0 and h0 == 0  =>  h1 = tanh(U_0) ---
            u0 = u_sb.rearrange("p (m s b) -> p m (s b)", m=KT, s=S)[:, :, 0:BL]
            if n_steps == 1:
                nc.scalar.activation(hF.rearrange("p (m b) -> p m b", m=KT), u0, Tanh)
            h_prev = ph.tile([128, W], f16, tag="h")
            nc.scalar.activation(h_prev.rearrange("p (m b) -> p m b", m=KT), u0, Tanh)

            # psum tiles are bank-bloated ([128,512] f32 = one full 2KB bank)
            # so each quarter's WAR/RAW tracking is per-tile and start=True's
            # whole-bank zero-marking can't touch a live sibling tile.
            with tc.tile_pool(name="pz", bufs=4, space="PSUM") as pz, \
                 tc.tile_pool(name="pzr", bufs=4, space="PSUM") as pzr:
                for s in range(1, n_steps):
                    h0 = h_prev
                    # z quarter-tiles accumulate across the 4 Euler sub-steps;
                    # each is zero-initialized by a start=True identity matmul
                    # (whole-bank ZERO_REGION marking makes per-slice start
                    # flags unsafe).
                    zq = [pz.tile([128, 512], f32, tag="z", name=f"z{s}_{q}") for q in range(4)]
                    for q in range(4):
                        nc.tensor.matmul(
                            zq[q][:, 0:QW], lhsT=id_sb[:, :], rhs=z0_sb[:, 0:QW],
                            start=True, stop=False, skip_group_check=True,
                        )
                    cur = h0      # matmul rhs for this sub-step
                    ha = h0       # h_s + dth*sum(t) accumulator
                    for e in range(N_ODE):
                        w_sb = wo_sb if e == 0 else wo2_sb
                        for i, (m, k) in enumerate(EORDERS[e]):
                            q = m // 2
                            nc.tensor.matmul(
                                zq[q][:, (m % 2) * BL:(m % 2) * BL + BL],
                                lhsT=w_sb[:, (k * KT + m) * 128:(k * KT + m) * 128 + 128],
                                rhs=cur[:, k * BL:(k + 1) * BL],
                                start=False,
                                stop=(e == N_ODE - 1 and i == e_last[m]),
                                skip_group_check=True,
                            )
                        t_e = pt.tile([128, W], f16, tag="t", name=f"t{s}_{e}")
                        for q in ACT_E:
                            nc.scalar.activation(t_e[:, QS[q]], zq[q][:, 0:QW], Tanh)
                        ha_new = pa.tile([128, W], f16, tag="ha", name=f"ha{s}_{e}")
                        for q in ACT_E:
                            nc.vector.scalar_tensor_tensor(
                                ha_new[:, QS[q]], t_e[:, QS[q]], dth, ha[:, QS[q]], op0=mult, op1=add
                            )
                        cur, ha = t_e, ha_new
                    # RNN block: zr = U_s + W_h @ h4 (h4 == ha); tile q is
                    # initialized with U_s quarter q via a start=True identity
                    # matmul, then W_h slice matmuls accumulate on top.
                    zrq = [pzr.tile([128, 512], f32, tag="zr", name=f"zr{s}_{q}") for q in range(4)]
                    us = u_sb.rearrange("p (m s b) -> p m s b", m=KT, s=S)[:, :, s, :]
                    for q in range(4):
                        nc.tensor.matmul(
                            zrq[q][:, 0:QW], lhsT=id_sb[:, :], rhs=us[:, 2 * q:2 * q + 2, :],
                            start=True, stop=False, skip_group_check=True,
                        )
                    for i, (m, k) in enumerate(RORDER):
                        q = m // 2
                        nc.tensor.matmul(
                            zrq[q][:, (m % 2) * BL:(m % 2) * BL + BL],
                            lhsT=wh_sb[:, (k * KT + m) * 128:(k * KT + m) * 128 + 128],
                            rhs=ha[:, k * BL:(k + 1) * BL],
                            start=False,
                            stop=(i == r_last[m]),
                            skip_group_check=True,
                        )
                    if s == n_steps - 1:
                        for q in ACT_R:
                            nc.scalar.activation(hF[:, QS[q]], zrq[q][:, 0:QW], Tanh)
                    else:
                        h_prev = ph.tile([128, W], f16, tag="h", name=f"hp{s}")
                        for q in ACT_R:
                            nc.scalar.activation(h_prev[:, QS[q]], zrq[q][:, 0:QW], Tanh)

            nc.sync.dma_start(out_d, hF)

    n_split = legalize_sync_waits(nc)
    print(f"legalize_sync_waits: split {n_split} instructions")
    return nc


def prep_inputs(x, t, W_in, b_in, W_h, b_h, W_ode, b_ode, n_steps=S):
    """Host-side prep: transpose/tile/cast; returns (in_maps, dths, with_bias)."""
    f16 = np.float16
    t = np.asarray(t, np.float32)
    t_prev = np.concatenate([t[:1], t[:-1]])
    dths = ((t - t_prev) / N_ODE).astype(np.float32)

    def tile_wT(W, ki):  # W: [H_out, K_in] -> [128, ki*8*128]
        return np.ascontiguousarray(
            W.T.reshape(ki, 128, KT, 128).transpose(1, 0, 2, 3).reshape(128, ki * KT * 128)
        ).astype(f16)

    wo = tile_wT(np.asarray(W_ode, np.float32), KT)
    wh = tile_wT(np.asarray(W_h, np.float32), KT)
    wi = tile_wT(np.asarray(W_in, np.float32), KI)
    nz = dths[1:n_steps] if n_steps > 1 else dths[1:1]
    uniform = len(nz) == 0 or (
        nz[0] != 0.0 and np.all(np.abs(nz - nz[0]) <= 1e-5 * abs(nz[0]))
    )
    wos = tile_wT(np.asarray(W_ode, np.float32) * (nz[0] if len(nz) else 1.0), KT) if uniform else None

    with_bias = not (
        np.all(np.asarray(b_in) == 0) and np.all(np.asarray(b_h) == 0) and np.all(np.asarray(b_ode) == 0)
    )

    in_maps = []
    for c in range(NCORES):
        xc = np.asarray(x[c * BL:(c + 1) * BL], np.float32)  # [BL, S, I]
        xt = (
            xc.transpose(2, 1, 0)  # [I, S, BL]
            .reshape(KI, 128, S, BL)
            .transpose(1, 0, 2, 3)
            .reshape(128, KI * S * BL)
        ).astype(f16)
        m = {"wo": wo, "wh": wh, "wi": wi, "xt": np.ascontiguousarray(xt)}
        if with_bias:
            bode = np.asarray(b_ode, np.float32).reshape(KT, 128)  # [m, p]
            bb = np.repeat(bode.T[:, :, None], BL, axis=2).reshape(128, KT * BL)
            m["bode"] = np.ascontiguousarray(bb.astype(np.float32))
            binh = (np.asarray(b_in, np.float32) + np.asarray(b_h, np.float32)).reshape(KT, 128).T
            m["binh"] = np.ascontiguousarray(binh.astype(np.float32))
        in_maps.append(m)
    return in_maps, dths, with_bias, uniform


def kernel(x, t, W_in, b_in, W_h, b_h, W_ode, b_ode):
    if "/opt/trn_rl_repo" not in sys.path:
        sys.path.insert(0, "/opt/trn_rl_repo")
    from concourse.bass_utils import run_bass_kernel_spmd

    in_maps, dths, with_bias, uniform = prep_inputs(
        x, t, W_in, b_in, W_h, b_h, W_ode, b_ode
    )
    if with_bias or not uniform:
        nc = build(dths, n_steps=S, with_bias=with_bias)
    else:
        nc = build_fast(dths, n_steps=S)
        ident = np.eye(128, dtype=np.float16)
        for m in in_maps:
            m["ident"] = ident

    res = run_bass_kernel_spmd(nc, in_maps, core_ids=list(range(NCORES)))

    outs = []
    for r in res.results:
        hf = r["hout"]  # [128, KT*BL]
        hT = hf.reshape(128, KT, BL).transpose(1, 0, 2).reshape(H, BL)
        outs.append(hT.T)
    return np.concatenate(outs, axis=0).astype(np.float32)



# revision 2
# speedup vs baseline: 1.0943x; 1.0943x over previous
"""ODE-RNN on Trainium2 (Bass/Tile), data-parallel over batch on 8 NeuronCores.

Strategy (per core, batch slice of 32, everything SBUF-resident):
  - h kept transposed: h_sb[p, 32k+b] = h[b, 128k+p]  ([128, 256] f16)
  - weights host-pretransposed+tiled so stationary tile (k,m) is
    w_sb[:, (k*8+m)*128 : +128] and psum[:, m-group] += tile.T @ h_k
  - U = x @ W_in.T (+ b_in + b_h) precomputed on-device for all timesteps
    (h-independent), consumed per-step by the RNN update
  - timestep 0 shortcut: dts[0] == 0 and h0 == 0 imply h after step 0 is
    tanh(U_0) for any inputs, so its 5 matmul blocks are skipped
  - fp16 operands with fp32 PSUM accumulation; all-zero-bias fast path
    (build_fast, v5 PSUM-resident z recurrence — see its docstring),
    general path with biases / non-uniform dt (build)

The recurrent phase is weight-load bound: each [128,128] f16 stationary
tile load sustains ~25ns back-to-back with its N=32 matmul, so a 64-MM
block floors at ~1.6us and the whole kernel at ~540us. v5 reaches toward
that floor by keeping tanh the only op on the cross-block critical path
and staggering psum-bank completion so ACT overlaps the next block.
"""

import sys

import numpy as np

B, S, I, H, N_ODE = 256, 64, 256, 1024, 4
NCORES = 8
BL = B // NCORES  # 32
KT = H // 128  # 8
KI = I // 128  # 2


def legalize_sync_waits(nc, max_waits=1):
    """This container's walrus rejects instructions carrying more than one
    sync-wait ("Too many sync wait commands", setupSyncWait). Hoist excess
    waits onto same-engine nop carriers inserted right before the offender."""
    n_split = 0
    for f in nc.m.functions:
        for bb in f.blocks:
            lst = bb.instructions
            i = 0
            while i < len(lst):
                inst = lst[i]
                si = inst.sync_info
                waits = list(si.on_wait) if (si and si.on_wait) else []
                if len(waits) > max_waits:
                    n_split += 1
                    keep = waits[-max_waits:]
                    hoist = waits[:-max_waits]
                    si.on_wait = keep
                    inst.sync_info = si
                    for w in hoist:
                        nop = nc.engines[inst.engine].nop(nofuse=True)
                        nsi = nop.ins.sync_info
                        if nsi is None:
                            import bass_rust
                            nsi = bass_rust.SyncInfo(on_wait=[w], on_update=[])
                        else:
                            nsi.on_wait = [w]
                        nop.ins.sync_info = nsi
                        # emission appended it to nc.cur_bb's list; relocate
                        src = nc.cur_bb.bb.instructions
                        assert src[-1].name == nop.ins.name
                        src.pop()
                        lst.insert(i, nop.ins)
                        i += 1
                i += 1
    return n_split


def build(dths, n_steps=S, with_bias=False):
    """Build the Bass module. dths: list of n_steps floats (dt/N_ODE per step)."""
    import concourse.bass as bass
    import concourse.tile as tile
    from concourse import mybir

    f16 = mybir.dt.float16
    f32 = mybir.dt.float32
    Tanh = mybir.ActivationFunctionType.Tanh
    Ident = mybir.ActivationFunctionType.Identity
    mult = mybir.AluOpType.mult
    add = mybir.AluOpType.add

    nc = bass.Bass("TRN2", target_bir_lowering=False, debug=False)

    wo_d = nc.dram_tensor("wo", [128, KT * KT * 128], f16, kind="ExternalInput").ap()
    wh_d = nc.dram_tensor("wh", [128, KT * KT * 128], f16, kind="ExternalInput").ap()
    wi_d = nc.dram_tensor("wi", [128, KI * KT * 128], f16, kind="ExternalInput").ap()
    xt_d = nc.dram_tensor("xt", [128, KI * S * BL], f16, kind="ExternalInput").ap()
    out_d = nc.dram_tensor("hout", [128, KT * BL], f32, kind="ExternalOutput").ap()
    if with_bias:
        bode_d = nc.dram_tensor("bode", [128, KT * BL], f32, kind="ExternalInput").ap()
        binh_d = nc.dram_tensor("binh", [128, KT], f32, kind="ExternalInput").ap()

    wo_sb = nc.alloc_sbuf_tensor("wo_sb", [128, KT * KT * 128], f16).ap()
    wh_sb = nc.alloc_sbuf_tensor("wh_sb", [128, KT * KT * 128], f16).ap()
    wi_sb = nc.alloc_sbuf_tensor("wi_sb", [128, KI * KT * 128], f16).ap()
    xt_sb = nc.alloc_sbuf_tensor("xt_sb", [128, KI * S * BL], f16).ap()
    u_sb = nc.alloc_sbuf_tensor("u_sb", [128, KT * S * BL], f16).ap()
    hA = nc.alloc_sbuf_tensor("hA", [128, KT * BL], f16).ap()
    hB = nc.alloc_sbuf_tensor("hB", [128, KT * BL], f16).ap()
    hF = nc.alloc_sbuf_tensor("hF", [128, KT * BL], f32).ap()
    if with_bias:
        bode_sb = nc.alloc_sbuf_tensor("bode_sb", [128, KT * BL], f32).ap()
        binh_sb = nc.alloc_sbuf_tensor("binh_sb", [128, KT], f32).ap()

    SB = S * BL  # 2048 (s,b) columns per k2

    with tile.TileContext(nc) as tc:
        with (
            tc.tile_pool(name="ppre", bufs=2, space="PSUM") as ppre,
            tc.tile_pool(name="pmain", bufs=4, space="PSUM") as pmain,
            tc.tile_pool(name="ptz", bufs=3) as ptz,
            tc.tile_pool(name="pz", bufs=3) as pz,
        ):
            # input DMAs (precompute deps first)
            nc.sync.dma_start(wi_sb, wi_d)
            nc.sync.dma_start(xt_sb, xt_d)
            if with_bias:
                nc.sync.dma_start(binh_sb, binh_d)
                nc.sync.dma_start(bode_sb, bode_d)
            nc.sync.dma_start(wo_sb, wo_d)
            nc.sync.dma_start(wh_sb, wh_d)

            # --- U precompute: U = x @ W_in.T (+ b_in + b_h) over all (s,b) ---
            NCHUNK = 4
            CW = SB // NCHUNK  # 512
            for m in range(KT):
                for c in range(NCHUNK):
                    ps = ppre.tile([128, CW], f32)
                    for k2 in range(KI):
                        nc.tensor.matmul(
                            ps[:, :],
                            lhsT=wi_sb[:, (k2 * KT + m) * 128:(k2 * KT + m) * 128 + 128],
                            rhs=xt_sb[:, k2 * SB + c * CW: k2 * SB + (c + 1) * CW],
                            start=(k2 == 0),
                            stop=(k2 == KI - 1),
                        )
                    dst = u_sb[:, m * SB + c * CW: m * SB + (c + 1) * CW]
                    if with_bias:
                        if (m * NCHUNK + c) % 2 == 0:
                            nc.scalar.activation(dst, ps[:, :], Ident, bias=binh_sb[:, m:m + 1])
                        else:
                            nc.vector.tensor_scalar_add(dst, ps[:, :], binh_sb[:, m:m + 1])
                    else:
                        if (m * NCHUNK + c) % 2 == 0:
                            nc.scalar.copy(dst, ps[:, :])
                        else:
                            nc.vector.tensor_copy(dst, ps[:, :])

            # --- timestep 0: dts[0]=0 and h0=0  =>  h1 = tanh(U_0) ---
            u0 = u_sb.rearrange("p (m s b) -> p m (s b)", m=KT, s=S)[:, :, 0:BL]
            hA3 = hA.rearrange("p (m b) -> p m b", m=KT)
            if n_steps == 1:
                nc.scalar.activation(hF.rearrange("p (m b) -> p m b", m=KT), u0, Tanh)
            else:
                nc.scalar.activation(hA3, u0, Tanh)

            cur, nxt = hA, hB

            def mm_block(ps, w_sb, hin):
                for m in range(KT):
                    for k in range(KT):
                        nc.tensor.matmul(
                            ps[:, m * BL:(m + 1) * BL],
                            lhsT=w_sb[:, (k * KT + m) * 128:(k * KT + m) * 128 + 128],
                            rhs=hin[:, k * BL:(k + 1) * BL],
                            start=(k == 0),
                            stop=(k == KT - 1),
                        )

            for s in range(1, n_steps):
                dth = float(dths[s])
                # 4 Euler sub-steps
                for e in range(N_ODE):
                    ps = pmain.tile([128, KT * BL], f32)
                    mm_block(ps, wo_sb, cur)
                    tz = ptz.tile([128, KT * BL], f16)
                    if with_bias:
                        zb = pz.tile([128, KT * BL], f32)
                        nc.vector.tensor_add(zb[:, :], ps[:, :], bode_sb)
                        nc.scalar.activation(tz[:, :], zb[:, :], Tanh)
                    else:
                        nc.scalar.activation(tz[:, :], ps[:, :], Tanh)
                    nc.vector.scalar_tensor_tensor(nxt, tz[:, :], dth, cur, op0=mult, op1=add)
                    cur, nxt = nxt, cur
                # RNN update
                ps = pmain.tile([128, KT * BL], f32)
                mm_block(ps, wh_sb, cur)
                z = pz.tile([128, KT * BL], f32)
                us = u_sb.rearrange("p (m s b) -> p m s b", m=KT, s=S)[:, :, s, :]
                nc.vector.tensor_add(
                    z.rearrange("p (m b) -> p m b", m=KT), ps.rearrange("p (m b) -> p m b", m=KT), us
                )
                if s == n_steps - 1:
                    nc.scalar.activation(hF, z[:, :], Tanh)
                else:
                    nc.scalar.activation(nxt, z[:, :], Tanh)
                    cur, nxt = nxt, cur

            nc.sync.dma_start(out_d, hF)

    n_split = legalize_sync_waits(nc)
    print(f"legalize_sync_waits: split {n_split} instructions")
    return nc


# MM emission priorities (m, k) per Euler sub-step, plus the RNN block's.
# The Tile scheduler re-sorts by virtual-cost readiness (its matmul cost
# omits LDWEIGHTS so the virtual PE runs ~2x fast); these priorities were
# tuned against a steady-state pipeline model and measured best on HW
# (748.8us total vs 766.8 baseline; plain m-outer 750.4, TILE_SCHEDULER=
# asap with in-order-optimal orders 1025.6 - asap convoys the pipeline).
# MM emission priorities (m, k) per Euler sub-step, plus the RNN block's.
# The Tile scheduler re-sorts by virtual-cost readiness (its matmul cost
# omits LDWEIGHTS so the virtual PE runs ~2x fast); these priorities were
# tuned against a steady-state pipeline model and measured best on HW
# (747.8us total vs 766.8 baseline; plain m-outer 750.4; 2-half-tanh
# granularity 795.3; TILE_SCHEDULER=asap with in-order-optimal orders
# 1025.6 - asap convoys the pipeline).
_EO = [(0,0),(1,0),(3,2),(3,3),(3,1),(0,1),(3,0),(1,1),(2,1),(2,0),(0,2),(2,3),(0,3),(2,2),(1,2),(4,5),(5,0),(5,4),(5,3),(3,5),(0,4),(2,4),(0,5),(1,5),(1,4),(2,5),(1,3),(1,7),(0,7),(0,6),(1,6),(4,4),(3,6),(5,2),(2,7),(6,5),(2,6),(7,0),(6,0),(6,2),(3,4),(5,1),(3,7),(7,1),(4,0),(7,7),(5,7),(5,5),(6,3),(4,7),(4,6),(7,3),(4,1),(4,3),(5,6),(4,2),(7,5),(7,6),(7,2),(6,1),(6,4),(6,7),(6,6),(7,4)]
EORDERS = [_EO, _EO, _EO, _EO]
RORDER = [(3,1),(3,0),(0,1),(0,0),(5,1),(2,1),(1,1),(5,0),(4,0),(4,1),(0,2),(2,2),(6,1),(6,0),(0,3),(1,0),(7,0),(7,3),(1,2),(2,3),(3,3),(1,3),(2,5),(4,4),(2,4),(1,4),(0,4),(7,2),(6,4),(3,2),(0,5),(1,5),(3,5),(3,4),(0,7),(1,6),(0,6),(1,7),(2,0),(3,6),(7,5),(5,5),(5,6),(3,7),(6,5),(4,5),(4,6),(2,6),(5,7),(2,7),(4,2),(5,4),(6,3),(6,6),(7,6),(7,1),(6,2),(5,3),(4,7),(4,3),(5,2),(7,7),(6,7),(7,4)]
ACT_E = [0, 1, 2, 3]
ACT_R = [0, 1, 2, 3]


def build_fast(dths, n_steps=S):
    import kernel_v6
    return kernel_v6.build_fast_v6(dths, n_steps=n_steps)


def build_fast_v5(dths, n_steps=S):
    """v5 zero-bias fast path: PSUM-resident z recurrence.

    Per timestep, the Euler chain is reassociated so PSUM accumulates z
    across sub-steps: z_0 = W_ode h_s; z_{e+1} = z_e + (dth*W_ode) t_e with
    t_e = tanh(z_e) read straight out of PSUM by ACT. This removes the DVE
    axpy from the critical path (h_4 = h_s + dth*sum(t_e) accumulates on DVE
    off-path). dth*W_ode is scaled on-chip (DVE) from the DMA'd W_ode.
    MMs are emitted in EORDER/RORDER so psum-bank completions stagger and
    each quarter's tanh overlaps the next block's matmuls. The RNN update
    injects U_s with a single N=256 identity matmul, then accumulates
    W_h h_4 on top. U precompute is chunk-major against split xt DMAs so
    the PE starts as soon as the first x chunk lands.
    """
    import os

    # Use the default CoreSim-cost Tile scheduler. (TILE_SCHEDULER=asap and
    # in-order-optimized emission were tried and measured SLOWER on HW:
    # asap re-derives its own earliest-feasible order from the same virtual
    # cost model and convoys the tanh/matmul pipeline.)
    os.environ.pop("TILE_SCHEDULER", None)

    import concourse.bass as bass
    import concourse.tile as tile
    from concourse import mybir

    f16 = mybir.dt.float16
    f32 = mybir.dt.float32
    Tanh = mybir.ActivationFunctionType.Tanh
    mult = mybir.AluOpType.mult
    add = mybir.AluOpType.add

    dth = float(dths[1]) if n_steps > 1 else 0.0

    nc = bass.Bass("TRN2", target_bir_lowering=False, debug=False)

    wo_d = nc.dram_tensor("wo", [128, KT * KT * 128], f16, kind="ExternalInput").ap()
    wh_d = nc.dram_tensor("wh", [128, KT * KT * 128], f16, kind="ExternalInput").ap()
    wi_d = nc.dram_tensor("wi", [128, KI * KT * 128], f16, kind="ExternalInput").ap()
    xt_d = nc.dram_tensor("xt", [128, KI * S * BL], f16, kind="ExternalInput").ap()
    id_d = nc.dram_tensor("ident", [128, 128], f16, kind="ExternalInput").ap()
    out_d = nc.dram_tensor("hout", [128, KT * BL], f32, kind="ExternalOutput").ap()

    wo_sb = nc.alloc_sbuf_tensor("wo_sb", [128, KT * KT * 128], f16).ap()
    wo2_sb = nc.alloc_sbuf_tensor("wo2_sb", [128, KT * KT * 128], f16).ap()
    wh_sb = nc.alloc_sbuf_tensor("wh_sb", [128, KT * KT * 128], f16).ap()
    wi_sb = nc.alloc_sbuf_tensor("wi_sb", [128, KI * KT * 128], f16).ap()
    xt_sb = nc.alloc_sbuf_tensor("xt_sb", [128, KI * S * BL], f16).ap()
    id_sb = nc.alloc_sbuf_tensor("id_sb", [128, 128], f16).ap()
    u_sb = nc.alloc_sbuf_tensor("u_sb", [128, KT * S * BL], f16).ap()
    z0_sb = nc.alloc_sbuf_tensor("z0_sb", [128, KT * BL], f16).ap()
    hF = nc.alloc_sbuf_tensor("hF", [128, KT * BL], f32).ap()

    SB = S * BL  # 2048
    W = KT * BL  # 256
    QW = W // 4  # 64
    QS = [slice(q * QW, (q + 1) * QW) for q in range(4)]

    # PSUM start=True zeroes the whole 2KB bank region (ZERO_REGION), so a
    # per-slice start flag clobbers sibling slices' accumulation. Instead each
    # psum z tile is initialized by ONE full-width start=True matmul (identity
    # x zeros / identity x U_s) and every slice matmul accumulates start=False.
    e_last = {m: max(i for i, (mm, _) in enumerate(EORDERS[N_ODE - 1]) if mm == m) for m in range(KT)}
    r_last = {m: max(i for i, (mm, _) in enumerate(RORDER) if mm == m) for m in range(KT)}

    with tile.TileContext(nc) as tc:
        with (
            tc.tile_pool(name="pt", bufs=6) as pt,
            tc.tile_pool(name="ph", bufs=4) as ph,
            tc.tile_pool(name="pa", bufs=6) as pa,
        ):
            nc.sync.dma_start(wi_sb, wi_d)
            # xt split chunk-major so U MMs start after the first slice lands
            NCHUNK = 4
            CW = SB // NCHUNK  # 512
            for c in range(NCHUNK):
                for k2 in range(KI):
                    off = k2 * SB + c * CW
                    nc.sync.dma_start(xt_sb[:, off:off + CW], xt_d[:, off:off + CW])
            nc.sync.dma_start(id_sb, id_d)
            nc.sync.dma_start(wo_sb, wo_d)
            nc.sync.dma_start(wh_sb, wh_d)
            nc.vector.memset(z0_sb, 0.0)

            # --- U = x @ W_in.T for all (s, b), chunk-major ---
            # wo2 = dth * wo scaling (DVE) is emitted mid-U: with in-order
            # (asap) emission, DVE ops placed earlier would block U's vector
            # copies behind the late wo DMA.
            NSC = 8
            SCW = KT * KT * 128 // NSC
            with tc.tile_pool(name="ppre", bufs=2, space="PSUM") as ppre:
                for c in range(NCHUNK):
                    if c == 2:
                        for j in range(NSC):
                            nc.vector.tensor_scalar_mul(
                                wo2_sb[:, j * SCW:(j + 1) * SCW], wo_sb[:, j * SCW:(j + 1) * SCW], dth
                            )
                    for m in range(KT):
                        ps = ppre.tile([128, CW], f32)
                        for k2 in range(KI):
                            nc.tensor.matmul(
                                ps[:, :],
                                lhsT=wi_sb[:, (k2 * KT + m) * 128:(k2 * KT + m) * 128 + 128],
                                rhs=xt_sb[:, k2 * SB + c * CW: k2 * SB + (c + 1) * CW],
                                start=(k2 == 0),
                                stop=(k2 == KI - 1),
                            )
                        dst = u_sb[:, m * SB + c * CW: m * SB + (c + 1) * CW]
                        if c >= 2 or (c * KT + m) % 2 == 0:
                            nc.scalar.copy(dst, ps[:, :])
                        else:
                            nc.vector.tensor_copy(dst, ps[:, :])

            # --- timestep 0: dts[0] == 0 and h0 == 0  =>  h1 = tanh(U_0) ---
            u0 = u_sb.rearrange("p (m s b) -> p m (s b)", m=KT, s=S)[:, :, 0:BL]
            if n_steps == 1:
                nc.scalar.activation(hF.rearrange("p (m b) -> p m b", m=KT), u0, Tanh)
            h_prev = ph.tile([128, W], f16, tag="h")
            nc.scalar.activation(h_prev.rearrange("p (m b) -> p m b", m=KT), u0, Tanh)

            # psum tiles are bank-bloated ([128,512] f32 = one full 2KB bank)
            # so each quarter's WAR/RAW tracking is per-tile and start=True's
            # whole-bank zero-marking can't touch a live sibling tile.
            with tc.tile_pool(name="pz", bufs=4, space="PSUM") as pz, \
                 tc.tile_pool(name="pzr", bufs=4, space="PSUM") as pzr:
                for s in range(1, n_steps):
                    h0 = h_prev
                    # z quarter-tiles accumulate across the 4 Euler sub-steps;
                    # each is zero-initialized by a start=True identity matmul
                    # (whole-bank ZERO_REGION marking makes per-slice start
                    # flags unsafe).
                    zq = [pz.tile([128, 512], f32, tag="z", name=f"z{s}_{q}") for q in range(4)]
                    for q in range(4):
                        nc.tensor.matmul(
                            zq[q][:, 0:QW], lhsT=id_sb[:, :], rhs=z0_sb[:, 0:QW],
                            start=True, stop=False, skip_group_check=True,
                        )
                    cur = h0      # matmul rhs for this sub-step
                    ha = h0       # h_s + dth*sum(t) accumulator
                    for e in range(N_ODE):
                        w_sb = wo_sb if e == 0 else wo2_sb
                        for i, (m, k) in enumerate(EORDERS[e]):
                            q = m // 2
                            nc.tensor.matmul(
                                zq[q][:, (m % 2) * BL:(m % 2) * BL + BL],
                                lhsT=w_sb[:, (k * KT + m) * 128:(k * KT + m) * 128 + 128],
                                rhs=cur[:, k * BL:(k + 1) * BL],
                                start=False,
                                stop=(e == N_ODE - 1 and i == e_last[m]),
                                skip_group_check=True,
                            )
                        t_e = pt.tile([128, W], f16, tag="t", name=f"t{s}_{e}")
                        for q in ACT_E:
                            nc.scalar.activation(t_e[:, QS[q]], zq[q][:, 0:QW], Tanh)
                        ha_new = pa.tile([128, W], f16, tag="ha", name=f"ha{s}_{e}")
                        for q in ACT_E:
                            nc.vector.scalar_tensor_tensor(
                                ha_new[:, QS[q]], t_e[:, QS[q]], dth, ha[:, QS[q]], op0=mult, op1=add
                            )
                        cur, ha = t_e, ha_new
                    # RNN block: zr = U_s + W_h @ h4 (h4 == ha); tile q is
                    # initialized with U_s quarter q via a start=True identity
                    # matmul, then W_h slice matmuls accumulate on top.
                    zrq = [pzr.tile([128, 512], f32, tag="zr", name=f"zr{s}_{q}") for q in range(4)]
                    us = u_sb.rearrange("p (m s b) -> p m s b", m=KT, s=S)[:, :, s, :]
                    for q in range(4):
                        nc.tensor.matmul(
                            zrq[q][:, 0:QW], lhsT=id_sb[:, :], rhs=us[:, 2 * q:2 * q + 2, :],
                            start=True, stop=False, skip_group_check=True,
                        )
                    for i, (m, k) in enumerate(RORDER):
                        q = m // 2
                        nc.tensor.matmul(
                            zrq[q][:, (m % 2) * BL:(m % 2) * BL + BL],
                            lhsT=wh_sb[:, (k * KT + m) * 128:(k * KT + m) * 128 + 128],
                            rhs=ha[:, k * BL:(k + 1) * BL],
                            start=False,
                            stop=(i == r_last[m]),
                            skip_group_check=True,
                        )
                    if s == n_steps - 1:
                        for q in ACT_R:
                            nc.scalar.activation(hF[:, QS[q]], zrq[q][:, 0:QW], Tanh)
                    else:
                        h_prev = ph.tile([128, W], f16, tag="h", name=f"hp{s}")
                        for q in ACT_R:
                            nc.scalar.activation(h_prev[:, QS[q]], zrq[q][:, 0:QW], Tanh)

            nc.sync.dma_start(out_d, hF)

    n_split = legalize_sync_waits(nc)
    print(f"legalize_sync_waits: split {n_split} instructions")
    return nc


def prep_inputs(x, t, W_in, b_in, W_h, b_h, W_ode, b_ode, n_steps=S):
    """Host-side prep: transpose/tile/cast; returns (in_maps, dths, with_bias)."""
    f16 = np.float16
    t = np.asarray(t, np.float32)
    t_prev = np.concatenate([t[:1], t[:-1]])
    dths = ((t - t_prev) / N_ODE).astype(np.float32)

    def tile_wT(W, ki):  # W: [H_out, K_in] -> [128, ki*8*128]
        return np.ascontiguousarray(
            W.T.reshape(ki, 128, KT, 128).transpose(1, 0, 2, 3).reshape(128, ki * KT * 128)
        ).astype(f16)

    wo = tile_wT(np.asarray(W_ode, np.float32), KT)
    wh = tile_wT(np.asarray(W_h, np.float32), KT)
    wi = tile_wT(np.asarray(W_in, np.float32), KI)
    nz = dths[1:n_steps] if n_steps > 1 else dths[1:1]
    uniform = len(nz) == 0 or (
        nz[0] != 0.0 and np.all(np.abs(nz - nz[0]) <= 1e-5 * abs(nz[0]))
    )
    wos = tile_wT(np.asarray(W_ode, np.float32) * (nz[0] if len(nz) else 1.0), KT) if uniform else None

    with_bias = not (
        np.all(np.asarray(b_in) == 0) and np.all(np.asarray(b_h) == 0) and np.all(np.asarray(b_ode) == 0)
    )

    in_maps = []
    for c in range(NCORES):
        xc = np.asarray(x[c * BL:(c + 1) * BL], np.float32)  # [BL, S, I]
        xt = (
            xc.transpose(2, 1, 0)  # [I, S, BL]
            .reshape(KI, 128, S, BL)
            .transpose(1, 0, 2, 3)
            .reshape(128, KI * S * BL)
        ).astype(f16)
        m = {"wo": wo, "wh": wh, "wi": wi, "xt": np.ascontiguousarray(xt)}
        if with_bias:
            bode = np.asarray(b_ode, np.float32).reshape(KT, 128)  # [m, p]
            bb = np.repeat(bode.T[:, :, None], BL, axis=2).reshape(128, KT * BL)
            m["bode"] = np.ascontiguousarray(bb.astype(np.float32))
            binh = (np.asarray(b_in, np.float32) + np.asarray(b_h, np.float32)).reshape(KT, 128).T
            m["binh"] = np.ascontiguousarray(binh.astype(np.float32))
        in_maps.append(m)
    return in_maps, dths, with_bias, uniform


def kernel(x, t, W_in, b_in, W_h, b_h, W_ode, b_ode):
    if "/opt/trn_rl_repo" not in sys.path:
        sys.path.insert(0, "/opt/trn_rl_repo")
    from concourse.bass_utils import run_bass_kernel_spmd

    in_maps, dths, with_bias, uniform = prep_inputs(
        x, t, W_in, b_in, W_h, b_h, W_ode, b_ode
    )
    if with_bias or not uniform:
        nc = build(dths, n_steps=S, with_bias=with_bias)
    else:
        nc = build_fast(dths, n_steps=S)
        ident = np.eye(128, dtype=np.float16)
        for m in in_maps:
            m["ident"] = ident

    res = run_bass_kernel_spmd(nc, in_maps, core_ids=list(range(NCORES)))

    outs = []
    for r in res.results:
        hf = r["hout"]  # [128, KT*BL]
        hT = hf.reshape(128, KT, BL).transpose(1, 0, 2).reshape(H, BL)
        outs.append(hT.T)
    return np.concatenate(outs, axis=0).astype(np.float32)

